# revision 30
# baseline (speedup 1.0000x reference)
"""Trainium2 Bass kernel for nn_PredictionModel (CPC-style prediction scores).

Reference computation (B=4, L=512, D=512, C=256, K=12, LW=500):
  cp[b,l,k,:]    = c[b,l,:] @ Wk[k].T            (row of R^D)
  zw[b,l,k,:]    = z[b, l+1+k, :]
  pos[b,l,k]     = <cp[b,l,k], zw[b,l,k]>
  neg_g[b,n,l,k] = <cp[b,l,k], zw[perm_B[n], perm_L[l], k]>
  neg_len[b,n,l,k]=<cp[b,l,k], zw[b, perms_len[n,l], k]>
  out = concat([pos[:,None], neg_g, neg_len], axis=1)   # (B, 9, LW, K)

Key algebraic move (C-space dots): <c[l] @ Wk[k].T, z[r]> = <c[l], z[r] @ Wk[k]>.
Define zp[r,k,:] = z[r,:] @ Wk[k] in R^C and the k-shifted table
zps[q,k,:] = zp[q+k,k,:]; then every score is
  score[...,l,k] = <c[b_out, l, :], zps_{b_src}[perm(l)+1, k, :]>
so one 6KB-contiguous row of zps serves all 12 k of a score row, and the
dot length is C=256 instead of D=512.

Per-core plan (8 cores = 4 source-batches x 2 q-halves):
  - PE computes zps_{b_src} for its 2 q-blocks directly in shifted layout
    (lhsT = z^T columns offset by k; 96 matmuls, bf16).
  - One dma_gather pulls the c rows (512B each) for all 9 units straight
    from HBM in q-natural order (indices baked host-side from the perms).
  - Dots: bf16 mul (c broadcast over k) + halving adds + reduce, split
    across DVE / ACT(accum) / Pool(fused stt) by a tunable mode string.
  - Scores are emitted q-indexed; the host un-permutes (pure indexing).
"""

import numpy as np
import ml_dtypes

import concourse.mybir as mybir
from concourse import bacc
from concourse.tile import TileContext
from concourse import bass_utils

B, L, D, C, K = 4, 512, 512, 256, 12
LW = L - K            # 500
NM = 2 * B + 1        # 9 output channels
NU = 9                # units per source batch
NQB = 2               # q-blocks per core
NI = NU * NQB * 128   # gather slots per core = 2304
COLS = NI // 16       # idx columns = 144
ZT_PAD = 272          # per-core window: 256 + 16 (k-shift slack)
F32 = mybir.dt.float32
BF16 = mybir.dt.bfloat16
I16 = mybir.dt.int16
BF16_NP = ml_dtypes.bfloat16

_NC = None

# engine mode per (unit, qj) flat index i = u*2+qj:
#   'd' = DVE mul+halve+halve+reduce
#   'a' = DVE mul+halve+halve, ACT per-k accum tail
#   'p' = Pool fused scalar_tensor_tensor per k
CFG = {
    "modes": "Ppadpadpadpadpaapd",
    "prod_bufs": 10,
    "junk_bufs": 4,
    "skew": 8,
    "warmup": 56,
}


def _build_program(cfg=None):
    """One NeuronCore program, identical across the 8 cores."""
    global _NC
    if cfg is None and _NC is not None:
        return _NC
    cfg = {**CFG, **(cfg or {})}
    modes = cfg["modes"]
    assert len(modes) == NU * NQB

    nc = bacc.Bacc()
    # z[b_src]^T padded: [128 d-part, 4 d-chunk, 528 r]
    zt_d = nc.dram_tensor("zt", [128, 4, ZT_PAD], BF16, kind="ExternalInput")
    # Wk transposed: [128 d-part, 4 d-chunk, K, C]
    wk_d = nc.dram_tensor("wk", [128, 4, K, C], BF16, kind="ExternalInput")
    # all batches' c rows: [B*L, C]
    call_d = nc.dram_tensor("call", [B * L, C], BF16, kind="ExternalInput")
    # gather index table (wrap-16 slots, replicated to all 8 Q7 groups)
    idx_d = nc.dram_tensor("idx", [128, COLS], I16, kind="ExternalInput")
    out_d = nc.dram_tensor("out", [128, NU, NQB, K], F32, kind="ExternalOutput")

    with TileContext(nc) as tc:
        with (
            tc.tile_pool(name="const", bufs=1) as const_pool,
            tc.tile_pool(name="psum", bufs=1, space="PSUM") as psum_pool,
            tc.tile_pool(name="prod", bufs=cfg["prod_bufs"]) as prod_pool,
            tc.tile_pool(name="half", bufs=cfg["prod_bufs"]) as half_pool,
            tc.tile_pool(name="junk", bufs=cfg["junk_bufs"]) as junk_pool,
        ):
            # PE p-state warmup: junk matmuls from t~0.8us keep the tensor
            # engine's busy-streak alive so real matmuls run at full clock.
            jmm = const_pool.tile([128, C], BF16, name="jmm")
            nc.gpsimd.memset(jmm[:], 0.0)
            psj = psum_pool.tile([128, C], F32, name="psj", tag="psj")
            for w in range(cfg.get("warmup", 52)):
                nc.tensor.matmul(
                    psj[:], jmm[:, :128], jmm[:], start=True, stop=True
                )

            idx_sb = const_pool.tile([128, COLS], I16, name="idx_sb")
            nc.scalar.dma_start(out=idx_sb[:], in_=idx_d[:])
            cg_sb = const_pool.tile([128, NU * NQB, C], BF16, name="cg_sb")
            nc.gpsimd.dma_gather(
                cg_sb[:], call_d[:], idx_sb[:], NI, NI, C, single_packet=False
            )
            zt_sb = const_pool.tile([128, 4, ZT_PAD], BF16, name="zt_sb")
            nc.sync.dma_start(out=zt_sb[:], in_=zt_d[:])
            # wk in two k-half tiles: PE's first psum group waits only for
            # the h=0 half (~4.4us of payload, one issue); the c-gather goes
            # last -- it isn't needed until the first mul.
            wk_sb = {}
            for h in range(2):
                t = const_pool.tile([128, 4, 6, C], BF16, name=f"wk{h}")
                eng = nc.sync if h == 0 else nc.scalar
                eng.dma_start(out=t[:], in_=wk_d[:, :, h * 6 : (h + 1) * 6])
                wk_sb[h] = t

            # zps for this core's two q-blocks, fold-friendly layout:
            # [128, qj, a, b, g, k, cc] with c' = a*128 + b*64 + g*32 + cc
            zps = {
                (qj, h): const_pool.tile(
                    [128, 2, 2, 2, 6, C // 8], BF16, name=f"zps{qj}_{h}"
                )
                for qj in range(NQB)
                for h in range(2)
            }
            for qj in range(NQB):
                for h in range(2):  # k-halves of 6
                    ps = psum_pool.tile(
                        [128, 6, C], F32, name=f"ps{qj}_{h}", tag=f"ps{(qj * 2 + h) % 2}"
                    )
                    for kk in range(6):
                        k = h * 6 + kk
                        for dc in range(4):
                            nc.tensor.matmul(
                                ps[:, kk],
                                zt_sb[:, dc, qj * 128 + k : qj * 128 + k + 128],
                                wk_sb[h][:, dc, kk, :],
                                start=(dc == 0),
                                stop=(dc == 3),
                            )
                    nc.scalar.copy(
                        zps[(qj, h)][:],
                        ps[:].rearrange(
                            "p k (a b g cc) -> p a b g k cc", a=2, b=2, g=2
                        ),
                    )

            scores = {
                (u, qj): const_pool.tile([128, K], F32, name=f"sc{u}_{qj}")
                for u in range(NU)
                for qj in range(NQB)
            }

            # software-pipelined emission: unit i's chain is emitted after
            # unit (i+SKEW)'s mul so no engine stalls on a cross-engine dep.
            SKEW = cfg.get("skew", 2)
            units = [(u, qj) for qj in range(NQB) for u in range(NU)]

            def emit_mul(u, qj):
                i = u * NQB + qj
                mode = modes[i]
                cgb = (
                    cg_sb[:, i, :]
                    .rearrange("p (a b g cc) -> p a b g cc", a=2, b=2, g=2)
                    .unsqueeze(4)
                    .broadcast_to([128, 2, 2, 2, 6, C // 8])
                )
                prod = prod_pool.tile(
                    [128, 2, 2, 2, K, C // 8], BF16, tag="pr", name=f"pr{i}"
                )
                mul_eng = nc.gpsimd if mode == "P" else nc.vector
                for h in range(2):
                    mul_eng.tensor_tensor(
                        out=prod[:, :, :, :, h * 6 : (h + 1) * 6, :],
                        in0=cgb,
                        in1=zps[(qj, h)][:],
                        op=mybir.AluOpType.mult,
                    )
                return prod

            def emit_chain(u, qj, prod):
                i = u * NQB + qj
                mode = modes[i]
                if mode == "a":
                    for k in range(K):
                        junk = junk_pool.tile(
                            [128, 2, 2, 2, C // 8], BF16, tag="aj",
                            name=f"aj{i}_{k}"
                        )
                        nc.scalar.activation(
                            out=junk[:],
                            in_=prod[:, :, :, :, k, :],
                            func=mybir.ActivationFunctionType.Copy,
                            accum_out=scores[(u, qj)][:, k : k + 1],
                        )
                    return
                fold_eng = nc.gpsimd if mode == "p" else nc.vector
                h1 = half_pool.tile(
                    [128, 2, 2, K, C // 8], BF16, tag="h1", name=f"h1{i}"
                )
                fold_eng.tensor_tensor(
                    out=h1[:], in0=prod[:, 0], in1=prod[:, 1],
                    op=mybir.AluOpType.add,
                )
                h2 = half_pool.tile(
                    [128, 2, K, C // 8], BF16, tag="h2", name=f"h2{i}"
                )
                fold_eng.tensor_tensor(
                    out=h2[:], in0=h1[:, 0], in1=h1[:, 1],
                    op=mybir.AluOpType.add,
                )
                h3 = half_pool.tile(
                    [128, K, C // 8], BF16, tag="h3", name=f"h3{i}"
                )
                fold_eng.tensor_tensor(
                    out=h3[:], in0=h2[:, 0], in1=h2[:, 1],
                    op=mybir.AluOpType.add,
                )
                nc.vector.tensor_reduce(
                    out=scores[(u, qj)][:], in_=h3[:],
                    axis=mybir.AxisListType.X, op=mybir.AluOpType.add,
                )
                return

            pending = []
            for u, qj in units:
                pending.append((u, qj, emit_mul(u, qj)))
                if len(pending) > SKEW:
                    pu, pq, pp = pending.pop(0)
                    emit_chain(pu, pq, pp)
            for pu, pq, pp in pending:
                emit_chain(pu, pq, pp)

            for u in range(NU):
                for qj in range(NQB):
                    nc.sync.dma_start(
                        out=out_d[:, u, qj], in_=scores[(u, qj)][:]
                    )

    nc.compile()
    if cfg == CFG:
        _NC = nc
    return nc


def _unit_perms(perms_len, perm_L, perm_B, b_src):
    """Per-unit (b_out, channel, forward-perm sl(l), inverse-perm l(sl))."""
    ident = np.arange(LW)
    inv_len = [np.argsort(perms_len[n]) for n in range(B)]
    inv_pl = np.argsort(perm_L)
    n_src = int(np.nonzero(perm_B == b_src)[0][0])
    units = [(b_src, 0, ident, ident)]
    for n in range(B):
        units.append((b_src, 1 + B + n, perms_len[n], inv_len[n]))
    for b_out in range(B):
        units.append((b_out, 1 + n_src, perm_L, inv_pl))
    return units


def _make_inputs(c, z, Wk, perms_len, perm_L, perm_B):
    """Host-side prep: transposed/padded operands + baked gather indices."""
    c_all = np.ascontiguousarray(c.reshape(B * L, C)).astype(BF16_NP)
    wk_dc = np.ascontiguousarray(
        Wk.reshape(K, 4, 128, C).transpose(2, 1, 0, 3)
    ).astype(BF16_NP)  # [128, 4, K, C]

    zt_full = np.zeros((B, 128, 4, L + 16), dtype=BF16_NP)
    for b in range(B):
        # zt[dp, dc, r] = z[b, r, dc*128+dp]
        zt = z[b].T.reshape(4, 128, L).transpose(1, 0, 2)  # [128, 4, 512]
        zt_full[b, :, :, :L] = zt.astype(BF16_NP)

    in_maps = []
    for b_src in range(B):
        units = _unit_perms(perms_len, perm_L, perm_B, b_src)
        for g in range(2):
            # zt window: program reads columns qj*128 + k + [0,128) for
            # qj in {0,1}; global q0 = g*256, so upload columns
            # [g*256, g*256 + 256 + 16) left-aligned.
            lo = g * 256
            ztw = np.ascontiguousarray(zt_full[b_src, :, :, lo : lo + ZT_PAD])

            idx = np.zeros((128, COLS), np.int16)
            s = np.arange(NQB * 128)
            for u, (b_out, _ch, _fwd, inv) in enumerate(units):
                q_glob = g * 256 + s
                sl = q_glob - 1
                valid = (sl >= 0) & (sl < LW)
                lvals = np.zeros(NQB * 128, np.int64)
                lvals[valid] = inv[sl[valid]]
                vals = np.where(valid, b_out * L + lvals, 0).astype(np.int16)
                # slot s_glob = u*256 + s -> idx[16*grp + s_glob%16, s_glob//16]
                col = u * 16 + s // 16
                row = s % 16
                for grp in range(8):
                    idx[16 * grp + row, col] = vals
            in_maps.append({"zt": ztw, "wk": wk_dc, "call": c_all, "idx": idx})
    return in_maps


def kernel(c, z, Wk, perms_len, perm_L, perm_B, _trace=False, _result_holder=None):
    c = np.asarray(c, np.float32)
    z = np.asarray(z, np.float32)
    Wk = np.asarray(Wk, np.float32)
    perms_len = np.asarray(perms_len, np.int64)
    perm_L = np.asarray(perm_L, np.int64)
    perm_B = np.asarray(perm_B, np.int64)

    nc = _build_program()
    in_maps = _make_inputs(c, z, Wk, perms_len, perm_L, perm_B)
    res = bass_utils.run_bass_kernel_spmd(
        nc, in_maps, core_ids=list(range(2 * B)), trace=_trace
    )
    if _result_holder is not None:
        _result_holder.append(res)

    out = np.empty((B, NM, LW, K), np.float32)
    larr = np.arange(LW)
    for b_src in range(B):
        units = _unit_perms(perms_len, perm_L, perm_B, b_src)
        for g in range(2):
            co = res.results[2 * b_src + g]["out"]  # [128, NU, NQB, K]
            for u, (b_out, ch, fwd, _inv) in enumerate(units):
                q = fwd + 1
                qb = q // 128
                sel = (qb // 2) == g
                out[b_out, ch, larr[sel]] = co[q[sel] % 128, u, qb[sel] % 2, :]
    return out
